# revision 22
# baseline (speedup 1.0000x reference)
"""Trainium2 Bass kernel for DifferentialEntropyRegularization (kNN loss).

reference math:
    dots = x @ x.T ; dots[i,i] = -1
    I = argmax(dots, axis=1)
    rho = ||x - x[I] + 1e-6||_2
    loss = -mean(log(rho + 1e-8))

Strategy (8 NeuronCores, fp8, fully local — no collectives):
  - each core gets the full x plus its own 1024-row query slab. It
    PE-transposes all of x to fp8 locally, slab by slab (collectives
    measured a ~65-85us barrier+transfer latency under this runner's
    skewed per-core launches, so local recompute wins), computing
    per-key norms on the fly.
  - scores s[k,q] = <x_k,x_q>_fp8 - (n_k-512)/2 via fp8 DoubleRow
    matmuls with keys on partitions; the norm correction is a
    per-partition bias applied for free on the PSUM->SBUF copy
    (split ACT/DVE).
  - nearest neighbor by squared distance: s_self - s_cross =
    ||x_q - x_k||^2 / 2, so rho^2 = 2*(m1 - m2) with the top-2 of s
    per query; m1 = n_q/2 + 256 exact in fp32.
  - selection: f16 running max per key-lane on DVE (ping-pong pair of
    accumulators, no in-place operands), then a small PE transpose +
    MAX8 recovers each query's top-2 across the 128 key lanes.
  - per-core partial sums of log(rho^2) reduced on host.
"""

import sys

sys.path.insert(0, "/opt/trn_rl_repo")

import numpy as np

import concourse.bass as bass
import concourse.mybir as mybir
import concourse.tile as tile
from concourse import bacc
from concourse.bass_utils import run_bass_kernel_spmd
from concourse.masks import make_identity

N = 8192
D = 512
NC = 8
SLAB = N // NC          # 1024 query rows per core
P = 128                 # partitions
QT = SLAB // P          # 8 row tiles per slab
KC = D // P             # 4 contraction chunks of 128

F32 = mybir.dt.float32
F16 = mybir.dt.float16
F8 = mybir.dt.float8e4
AF = mybir.ActivationFunctionType
ALU = mybir.AluOpType
DR = mybir.MatmulPerfMode.DoubleRow

# score copies: ACT when (idx % MOD) < THR, else DVE (same split v3 ran with)
ACT_COPY_MOD = 4
ACT_COPY_THR = 3

_cache = {}


def _build():
    nc = bacc.Bacc("TRN2", target_bir_lowering=False, debug=False, num_devices=NC)

    xq_d = nc.dram_tensor("xq", [SLAB, D], F32, kind="ExternalInput")
    x_d = nc.dram_tensor("x", [N, D], F32, kind="ExternalInput")
    part_d = nc.dram_tensor("partial", [1, 1], F32, kind="ExternalOutput")

    with tile.TileContext(nc) as tc:
        with (
            tc.tile_pool(name="const", bufs=1) as constp,
            tc.tile_pool(name="big", bufs=1) as bigp,
        ):
            identf = constp.tile([P, P], F32)
            make_identity(nc, identf[:])
            ones = constp.tile([P, 1], F32)
            nc.vector.memset(ones[:], 1.0)
            nq = constp.tile([P, QT], F32)
            negdn = constp.tile([P, QT], F32)

            xq_sb = bigp.tile([P, QT, D], F32)
            xTq = bigp.tile([P, KC, SLAB], F8)
            xTl = [bigp.tile([P, KC, SLAB], F8, name=f"xTl{n}") for n in range(NC)]
            biasl = [constp.tile([P, QT], F32, name=f"biasl{n}") for n in range(NC)]
            mb = [bigp.tile([P, SLAB], F16, name=f"mb{i}") for i in range(2)]

            with (
                tc.tile_pool(name="wpsum", bufs=1, space="PSUM") as wpsum,
                tc.tile_pool(name="small", bufs=1) as smallp,
                tc.tile_pool(name="xlp", bufs=2) as xlp,
            ):
                for qt in range(QT):
                    nc.sync.dma_start(
                        out=xq_sb[:, qt, :], in_=xq_d.ap()[qt * P : (qt + 1) * P]
                    )

                def slab_transpose(src_sb, dst_f8):
                    for qt in range(QT):
                        pt = wpsum.tile([P, KC * P], F32, tag="tr", bufs=2)
                        for kc in range(KC):
                            nc.tensor.transpose(
                                pt[:, kc * P : (kc + 1) * P],
                                src_sb[:, qt, kc * P : (kc + 1) * P],
                                identf[:],
                            )
                        nc.scalar.copy(
                            out=dst_f8[:, :, qt * P : (qt + 1) * P],
                            in_=pt[:].rearrange("p (kc q) -> p kc q", kc=KC),
                        )

                def slab_norms(src_sb, nq_out, negdn_out):
                    # nq = ||row||^2 (fp32), negdn = -(nq-512)/2
                    for qt in range(QT):
                        sq = smallp.tile([P, D], F32, tag="sq", bufs=2)
                        nc.scalar.activation(
                            out=sq[:], in_=src_sb[:, qt, :], func=AF.Square,
                            accum_out=nq_out[:, qt : qt + 1],
                        )
                    nc.vector.tensor_scalar(
                        negdn_out[:], nq_out[:], -0.5, 256.0,
                        op0=ALU.mult, op1=ALU.add,
                    )

                slab_norms(xq_sb, nq, negdn)
                slab_transpose(xq_sb, xTq)

                nc.vector.memset(mb[0][:], -10000.0)

                idx = 0

                def score_tiles(keys, bias):
                    nonlocal idx
                    for kt in range(QT):
                        pp = wpsum.tile([P, SLAB], F32, tag="pp", bufs=3)
                        for c2 in range(2):
                            for qh in range(2):
                                nc.tensor.matmul(
                                    pp[:, qh * 512 : (qh + 1) * 512],
                                    lhsT=keys[:, 2 * c2 : 2 * c2 + 2, kt * P : (kt + 1) * P],
                                    rhs=xTq[:, 2 * c2 : 2 * c2 + 2, qh * 512 : (qh + 1) * 512],
                                    start=(c2 == 0),
                                    stop=(c2 == 1),
                                    perf_mode=DR,
                                )
                        s16 = smallp.tile([P, SLAB], F16, tag="s16", bufs=6)
                        bap = bias[:, kt : kt + 1]
                        if (idx % ACT_COPY_MOD) < ACT_COPY_THR:
                            nc.scalar.add(s16[:], pp[:], bap)
                        else:
                            nc.vector.tensor_scalar_add(s16[:], pp[:], bap)
                        # ping-pong the running max: write the other buffer
                        nc.vector.tensor_tensor(
                            out=mb[(idx + 1) % 2][:],
                            in0=s16[:],
                            in1=mb[idx % 2][:],
                            op=ALU.max,
                        )
                        idx += 1

                # key slabs in absolute order (max is permutation-invariant,
                # so identical instructions work on every core)
                for n in range(NC):
                    xl_sb = xlp.tile([P, QT, D], F32, tag="xl")
                    eng = nc.sync if n % 2 == 0 else nc.gpsimd
                    for qt in range(QT):
                        eng.dma_start(
                            out=xl_sb[:, qt, :],
                            in_=x_d.ap()[n * SLAB + qt * P : n * SLAB + (qt + 1) * P],
                        )
                    nql = smallp.tile([P, QT], F32, tag="nql", bufs=2)
                    slab_norms(xl_sb, nql, biasl[n])
                    slab_transpose(xl_sb, xTl[n])
                    score_tiles(xTl[n], biasl[n])

                # 64 tiles: final running max lands in mb[64 % 2] = mb[0]
                mfin = mb[idx % 2]

                # ---- per-query top-2 across key lanes ----
                m32 = smallp.tile([P, SLAB], F32, tag="m32")
                nc.vector.tensor_copy(m32[:], mfin[:])
                ftr = wpsum.tile([P, SLAB], F32, tag="pp", bufs=3)
                for b in range(QT):
                    nc.tensor.transpose(
                        ftr[:, b * P : (b + 1) * P],
                        m32[:, b * P : (b + 1) * P],
                        identf[:],
                    )
                mt = smallp.tile([P, QT, P], F16, tag="mt")
                nc.scalar.copy(
                    out=mt[:], in_=ftr[:].rearrange("p (b q) -> p b q", b=QT)
                )
                gtop = smallp.tile([P, QT, 8], F16, tag="gtop")
                for b in range(QT):
                    nc.vector.max(out=gtop[:, b, :], in_=mt[:, b, :])

                # rho^2 = 2*(m1 - m2), m1 = 512 - negdn (exact fp32)
                m2_32 = smallp.tile([P, QT], F32, tag="m2")
                nc.vector.tensor_copy(
                    m2_32[:], gtop[:, :, 1:2].rearrange("p b r -> p (b r)")
                )
                m1f = smallp.tile([P, QT], F32, tag="m1")
                nc.vector.tensor_scalar(
                    m1f[:], negdn[:], -1.0, 512.0, op0=ALU.mult, op1=ALU.add
                )
                delta = smallp.tile([P, QT], F32, tag="delta")
                nc.vector.tensor_tensor(
                    out=delta[:], in0=m1f[:], in1=m2_32[:], op=ALU.subtract
                )
                logs = smallp.tile([P, QT], F32, tag="logs")
                nc.scalar.activation(
                    out=logs[:], in_=delta[:], func=AF.Ln, bias=0.0, scale=2.0
                )
                rowsum = smallp.tile([P, 1], F32, tag="rowsum")
                nc.vector.tensor_reduce(
                    rowsum[:], logs[:], axis=mybir.AxisListType.X, op=ALU.add
                )
                fin = wpsum.tile([1, 1], F32, tag="tr", bufs=2)
                nc.tensor.matmul(
                    fin[:], lhsT=rowsum[:], rhs=ones[:], start=True, stop=True
                )
                outsb = smallp.tile([1, 1], F32, tag="outsb")
                nc.scalar.copy(outsb[:], fin[:])
                nc.sync.dma_start(out=part_d.ap(), in_=outsb[:])

    nc.compile()
    return nc


def get_nc():
    if "nc" not in _cache:
        _cache["nc"] = _build()
    return _cache["nc"]


def run(x: np.ndarray, **spmd_kwargs):
    nc = get_nc()
    x = np.ascontiguousarray(x, dtype=np.float32)
    in_maps = [{"x": x, "xq": x[c * SLAB : (c + 1) * SLAB]} for c in range(NC)]
    res = run_bass_kernel_spmd(nc, in_maps, list(range(NC)), **spmd_kwargs)
    total = sum(float(res.results[c]["partial"][0, 0]) for c in range(NC))
    # partial = sum of log(rho^2) = sum of 2*log(rho)
    loss = np.float32(-0.5 * total / N)
    return np.asarray(loss, dtype=np.float32), res


def kernel(x: np.ndarray) -> np.ndarray:
    loss, _ = run(x)
    return loss


# revision 24
# speedup vs baseline: 1.0885x; 1.0885x over previous
"""Trainium2 Bass kernel for DifferentialEntropyRegularization (kNN loss).

reference math:
    dots = x @ x.T ; dots[i,i] = -1
    I = argmax(dots, axis=1)
    rho = ||x - x[I] + 1e-6||_2
    loss = -mean(log(rho + 1e-8))

Strategy (8 NeuronCores, fp8, fully local — no collectives):
  - each core gets the full x plus its own 1024-row query slab. It
    PE-transposes all of x to fp8 locally, slab by slab (collectives
    measured a ~65-85us barrier+transfer latency under this runner's
    skewed per-core launches, so local recompute wins), computing
    per-key norms on the fly.
  - scores s[k,q] = <x_k,x_q>_fp8 - (n_k-512)/2 via fp8 DoubleRow
    matmuls with keys on partitions; the norm correction is a
    per-partition bias applied for free on the PSUM->SBUF copy
    (split ACT/DVE).
  - nearest neighbor by squared distance: s_self - s_cross =
    ||x_q - x_k||^2 / 2, so rho^2 = 2*(m1 - m2) with the top-2 of s
    per query; m1 = n_q/2 + 256 exact in fp32.
  - selection: f16 running max per key-lane on DVE (ping-pong pair of
    accumulators, no in-place operands), then a small PE transpose +
    MAX8 recovers each query's top-2 across the 128 key lanes.
  - per-core partial sums of log(rho^2) reduced on host.
"""

import sys

sys.path.insert(0, "/opt/trn_rl_repo")

import numpy as np

import concourse.bass as bass
import concourse.mybir as mybir
import concourse.tile as tile
from concourse import bacc
from concourse.bass_utils import run_bass_kernel_spmd
from concourse.masks import make_identity

N = 8192
D = 512
NC = 8
SLAB = N // NC          # 1024 query rows per core
P = 128                 # partitions
QT = SLAB // P          # 8 row tiles per slab
KC = D // P             # 4 contraction chunks of 128

F32 = mybir.dt.float32
F16 = mybir.dt.float16
F8 = mybir.dt.float8e4
AF = mybir.ActivationFunctionType
ALU = mybir.AluOpType
DR = mybir.MatmulPerfMode.DoubleRow

# engine-balance knobs, set from the measured v4c trace (ACT saturated
# at 156us): ~44% of score copies and 1/3 of transpose copies on ACT,
# rest on DVE; squares stay on ACT.
ACT_COPY_MOD = 9
ACT_COPY_THR = 4
TRC_DVE_MOD = 3

_cache = {}


def _build():
    nc = bacc.Bacc("TRN2", target_bir_lowering=False, debug=False, num_devices=NC)

    xq_d = nc.dram_tensor("xq", [SLAB, D], F32, kind="ExternalInput")
    x_d = nc.dram_tensor("x", [N, D], F32, kind="ExternalInput")
    part_d = nc.dram_tensor("partial", [1, 1], F32, kind="ExternalOutput")

    with tile.TileContext(nc) as tc:
        with (
            tc.tile_pool(name="const", bufs=1) as constp,
            tc.tile_pool(name="big", bufs=1) as bigp,
        ):
            identf = constp.tile([P, P], F32)
            make_identity(nc, identf[:])
            ones = constp.tile([P, 1], F32)
            nc.vector.memset(ones[:], 1.0)
            nq = constp.tile([P, QT], F32)
            negdn = constp.tile([P, QT], F32)

            xq_sb = bigp.tile([P, QT, D], F32)
            xTq = bigp.tile([P, KC, SLAB], F8)
            xTl = [bigp.tile([P, KC, SLAB], F8, name=f"xTl{n}") for n in range(NC)]
            biasl = [constp.tile([P, QT], F32, name=f"biasl{n}") for n in range(NC)]
            mb = [bigp.tile([P, SLAB], F16, name=f"mb{i}") for i in range(2)]

            with (
                tc.tile_pool(name="wpsum", bufs=1, space="PSUM") as wpsum,
                tc.tile_pool(name="small", bufs=1) as smallp,
                tc.tile_pool(name="xlp", bufs=3) as xlp,
            ):
                for qt in range(QT):
                    nc.sync.dma_start(
                        out=xq_sb[:, qt, :], in_=xq_d.ap()[qt * P : (qt + 1) * P]
                    )

                def slab_transpose(src_sb, dst_f8):
                    for qt in range(QT):
                        pt = wpsum.tile([P, KC * P], F32, tag="tr", bufs=2)
                        for kc in range(KC):
                            nc.tensor.transpose(
                                pt[:, kc * P : (kc + 1) * P],
                                src_sb[:, qt, kc * P : (kc + 1) * P],
                                identf[:],
                            )
                        dst = dst_f8[:, :, qt * P : (qt + 1) * P]
                        srcap = pt[:].rearrange("p (kc q) -> p kc q", kc=KC)
                        if (qt % TRC_DVE_MOD) == TRC_DVE_MOD - 1:
                            nc.vector.tensor_copy(dst, srcap)
                        else:
                            nc.scalar.copy(out=dst, in_=srcap)

                def slab_norms(src_sb, nq_out, negdn_out):
                    # nq = ||row||^2 (fp32), negdn = -(nq-512)/2
                    for qt in range(QT):
                        sq = smallp.tile([P, D], F32, tag="sq", bufs=2)
                        nc.scalar.activation(
                            out=sq[:], in_=src_sb[:, qt, :], func=AF.Square,
                            accum_out=nq_out[:, qt : qt + 1],
                        )
                    nc.vector.tensor_scalar(
                        negdn_out[:], nq_out[:], -0.5, 256.0,
                        op0=ALU.mult, op1=ALU.add,
                    )

                slab_norms(xq_sb, nq, negdn)
                slab_transpose(xq_sb, xTq)

                nc.vector.memset(mb[0][:], -10000.0)

                idx = 0

                def score_tiles(keys, bias):
                    nonlocal idx
                    for kt in range(QT):
                        pp = wpsum.tile([P, SLAB], F32, tag="pp", bufs=3)
                        for c2 in range(2):
                            for qh in range(2):
                                nc.tensor.matmul(
                                    pp[:, qh * 512 : (qh + 1) * 512],
                                    lhsT=keys[:, 2 * c2 : 2 * c2 + 2, kt * P : (kt + 1) * P],
                                    rhs=xTq[:, 2 * c2 : 2 * c2 + 2, qh * 512 : (qh + 1) * 512],
                                    start=(c2 == 0),
                                    stop=(c2 == 1),
                                    perf_mode=DR,
                                )
                        s16 = smallp.tile([P, SLAB], F16, tag="s16", bufs=6)
                        bap = bias[:, kt : kt + 1]
                        if (idx % ACT_COPY_MOD) < ACT_COPY_THR:
                            nc.scalar.add(s16[:], pp[:], bap)
                        else:
                            nc.vector.tensor_scalar_add(s16[:], pp[:], bap)
                        # ping-pong the running max: write the other buffer
                        nc.vector.tensor_tensor(
                            out=mb[(idx + 1) % 2][:],
                            in0=s16[:],
                            in1=mb[idx % 2][:],
                            op=ALU.max,
                        )
                        idx += 1

                # key slabs in absolute order (max is permutation-invariant,
                # so identical instructions work on every core)
                for n in range(NC):
                    xl_sb = xlp.tile([P, QT, D], F32, tag="xl")
                    eng = nc.sync if n % 2 == 0 else nc.gpsimd
                    for qt in range(QT):
                        eng.dma_start(
                            out=xl_sb[:, qt, :],
                            in_=x_d.ap()[n * SLAB + qt * P : n * SLAB + (qt + 1) * P],
                        )
                    nql = smallp.tile([P, QT], F32, tag="nql", bufs=2)
                    slab_norms(xl_sb, nql, biasl[n])
                    slab_transpose(xl_sb, xTl[n])
                    score_tiles(xTl[n], biasl[n])

                # 64 tiles: final running max lands in mb[64 % 2] = mb[0]
                mfin = mb[idx % 2]

                # ---- per-query top-2 across key lanes ----
                m32 = smallp.tile([P, SLAB], F32, tag="m32")
                nc.vector.tensor_copy(m32[:], mfin[:])
                ftr = wpsum.tile([P, SLAB], F32, tag="pp", bufs=3)
                for b in range(QT):
                    nc.tensor.transpose(
                        ftr[:, b * P : (b + 1) * P],
                        m32[:, b * P : (b + 1) * P],
                        identf[:],
                    )
                mt = smallp.tile([P, QT, P], F16, tag="mt")
                nc.scalar.copy(
                    out=mt[:], in_=ftr[:].rearrange("p (b q) -> p b q", b=QT)
                )
                gtop = smallp.tile([P, QT, 8], F16, tag="gtop")
                for b in range(QT):
                    nc.vector.max(out=gtop[:, b, :], in_=mt[:, b, :])

                # rho^2 = 2*(m1 - m2), m1 = 512 - negdn (exact fp32)
                m2_32 = smallp.tile([P, QT], F32, tag="m2")
                nc.vector.tensor_copy(
                    m2_32[:], gtop[:, :, 1:2].rearrange("p b r -> p (b r)")
                )
                m1f = smallp.tile([P, QT], F32, tag="m1")
                nc.vector.tensor_scalar(
                    m1f[:], negdn[:], -1.0, 512.0, op0=ALU.mult, op1=ALU.add
                )
                delta = smallp.tile([P, QT], F32, tag="delta")
                nc.vector.tensor_tensor(
                    out=delta[:], in0=m1f[:], in1=m2_32[:], op=ALU.subtract
                )
                logs = smallp.tile([P, QT], F32, tag="logs")
                nc.scalar.activation(
                    out=logs[:], in_=delta[:], func=AF.Ln, bias=0.0, scale=2.0
                )
                rowsum = smallp.tile([P, 1], F32, tag="rowsum")
                nc.vector.tensor_reduce(
                    rowsum[:], logs[:], axis=mybir.AxisListType.X, op=ALU.add
                )
                fin = wpsum.tile([1, 1], F32, tag="tr", bufs=2)
                nc.tensor.matmul(
                    fin[:], lhsT=rowsum[:], rhs=ones[:], start=True, stop=True
                )
                outsb = smallp.tile([1, 1], F32, tag="outsb")
                nc.scalar.copy(outsb[:], fin[:])
                nc.sync.dma_start(out=part_d.ap(), in_=outsb[:])

    nc.compile()
    return nc


def get_nc():
    if "nc" not in _cache:
        _cache["nc"] = _build()
    return _cache["nc"]


def run(x: np.ndarray, **spmd_kwargs):
    nc = get_nc()
    x = np.ascontiguousarray(x, dtype=np.float32)
    in_maps = [{"x": x, "xq": x[c * SLAB : (c + 1) * SLAB]} for c in range(NC)]
    res = run_bass_kernel_spmd(nc, in_maps, list(range(NC)), **spmd_kwargs)
    total = sum(float(res.results[c]["partial"][0, 0]) for c in range(NC))
    # partial = sum of log(rho^2) = sum of 2*log(rho)
    loss = np.float32(-0.5 * total / N)
    return np.asarray(loss, dtype=np.float32), res


def kernel(x: np.ndarray) -> np.ndarray:
    loss, _ = run(x)
    return loss
